# revision 34
# baseline (speedup 1.0000x reference)
"""Trainium2 Bass kernel for spatial self-attention (nn_Attention_90615220011343).

Module math (per batch b):
    qkv = x @ w_qkv            x:[N=4096, C=256], w_qkv:[256, 384]
    q,k,v -> heads (4 heads, dim 32)
    sim = (q*ds^-0.5) @ k^T    per head: [4096, 4096]
    attn = softmax(sim, -1)
    out = attn @ v             -> [N, 128]
    y = out @ w_out + b_out    -> [N, 256]

Sharding: 8 cores = 4 batches x 2 head-pairs. Core c -> batch c//2,
heads {2*(c%2), 2*(c%2)+1}. Each core computes a partial y (its two
heads' contribution); host sums the pair and adds b_out.

Per-core layout (all on-chip, no collectives). The Activation engine is
the roofline (33.5M softmax exps / 128 lanes); everything else is
organized to keep it saturated:
  - x^T [2x128, 4096] via PE transposes (contraction dim C on partitions).
  - q^T, k^T stored flat [32, 4096] fp32r (contract dim 32 on partitions
    0-31; sim matmuls stream 512-col i-tiles at 1 cycle/row).
  - sim^T computed in [j, i] psum slabs (A: 4 banks / B: 3 banks,
    ping-pong) so exp is one big Activation op per slab, no reductions.
  - exp output in fp16; attn@v flipped to out[i, d]: lhsT = exp-slab
    [j, 128-i-block], rhs = [v_h | 1] fp16 [j, 33] -> 33-row matmuls
    accumulating [128, 33] per i-block in psum, denominator rides in
    column 32.  4x fewer PE rows than the [d, i] orientation.
  - per i-tile: reciprocal + scale (DVE), PE-transpose of the normalized
    [128i, 4x32d] block into outT [64, 4096] fp16.
  - y = outT-block^T @ w_out contracts both heads at once (fp16), riding
    the A-slab psum ring lagged one i-tile behind attention.
"""

import numpy as np

HEADS = 4
DH = 32
N = 4096
C = 256
P = 128
NCH = 32  # number of 128-token j-chunks
ITILES = 8  # i tiles of 512
# j-chunks per sim/exp group. The 1-chunk A group before the final B group
# keeps every same-slab pair separated by the other slab's exp, so sim
# refills always hide under an exp (no tile-boundary fill gap).
GROUPS = [4, 3, 4, 3, 4, 3, 4, 3, 1, 3]

_CACHED = {}


def _build_nc():
    import concourse.bass as bass
    import concourse.mybir as mybir
    from concourse.tile import TileContext
    from concourse.masks import make_identity

    FP = mybir.dt.float32
    FR = mybir.dt.float32r
    F16 = mybir.dt.float16
    AF = mybir.ActivationFunctionType

    nc = bass.Bass(target_bir_lowering=False)
    x_d = nc.declare_dram_parameter("x", [N, C], FP, isOutput=False)
    wq_d = nc.declare_dram_parameter("wq", [C, 64], FP, isOutput=False)
    wk_d = nc.declare_dram_parameter("wk", [C, 64], FP, isOutput=False)
    wv_d = nc.declare_dram_parameter("wv", [C, 64], FP, isOutput=False)
    wo_d = nc.declare_dram_parameter("wo", [64, C], FP, isOutput=False)
    y_d = nc.declare_dram_parameter("y", [N, C], FP, isOutput=True)

    with TileContext(nc) as tc:
        with (
            tc.tile_pool(name="const", bufs=1) as constp,
            tc.tile_pool(name="xin", bufs=1) as xinp,
            tc.tile_pool(name="big", bufs=1) as bigp,
            tc.tile_pool(name="exp", bufs=3) as expp,
            tc.tile_pool(name="stg", bufs=2) as stgp,
            tc.tile_pool(name="ytmp", bufs=2) as ytmpp,
            tc.tile_pool(name="psA", bufs=1, space="PSUM") as psA,
            tc.tile_pool(name="psB", bufs=1, space="PSUM") as psB,
            tc.tile_pool(name="psV", bufs=1, space="PSUM") as psV,
        ):
            ident = constp.tile([P, P], FP, tag="ident")
            make_identity(nc, ident[:])
            # preload the activation table so the first real Act op (x copy
            # or exp) doesn't pay the 1.3us table load mid-chain
            warm = constp.tile([P, 1], FP, tag="warm")
            nc.vector.memset(warm[:], 0.0)
            nc.scalar.activation(warm[:], warm[:], AF.Exp)

            # ---- persistent SBUF tensors ----
            xT = [bigp.tile([P, N], FR, tag=f"xT{cc}", name=f"xT{cc}") for cc in range(2)]
            qT = bigp.tile([64, N], FR, tag="qT")
            karr = bigp.tile([64, N], FR, tag="karr")
            vaug = [bigp.tile([P, 33 * NCH], F16, tag=f"vaug{h}", name=f"vaug{h}") for h in range(2)]
            outT = bigp.tile([64, N], F16, tag="outT")
            rden = bigp.tile([P, 8 * ITILES], FP, tag="rden")
            wq_sb = bigp.tile([P, 2, 64], FR, tag="wq")
            wk_sb = bigp.tile([P, 2, 64], FR, tag="wk")
            wv_sb = bigp.tile([P, 2, 64], FR, tag="wv")
            wo_sb = bigp.tile([64, C], F16, tag="wo")

            # ---- weight staging tiles (DMAs interleaved with x below) ----
            wq_st = bigp.tile([P, 2, 64], FP, tag="wq_st")
            wk_st = bigp.tile([P, 2, 64], FP, tag="wk_st")
            wv_st = bigp.tile([P, 2, 64], FP, tag="wv_st")
            wo_st = bigp.tile([64, C], FP, tag="wo_st")

            def wdma(st, d):
                nc.gpsimd.dma_start(
                    out=st[:], in_=d[:, :].rearrange("(c p) f -> p c f", p=P)
                )

            # ---- x load + transpose to xT; qkv builds interleaved ----
            # 8 batched DMAs (4 chunks each) into persistent staging tiles;
            # build generations are interleaved into the A/B psum rings so
            # each gen's ring predecessor matches its data dependencies.
            # karr copies ride the idle Act engine, qT/v copies ride DVE.
            xt4s = [
                xinp.tile([P, 4 * C], FP, tag=f"xt{b}", name=f"xt4_{b}")
                for b in range(8)
            ]

            def xdma(b):
                dmae = nc.sync if b % 2 == 0 else nc.gpsimd
                dmae.dma_start(
                    out=xt4s[b][:].rearrange("p (k c) -> p k c", c=C),
                    in_=x_d[512 * b: 512 * (b + 1), :].rearrange(
                        "(k p) c -> p k c", p=P
                    ),
                )

            for b in (0, 2, 4, 6):
                xdma(b)
            wdma(wq_st, wq_d)
            xdma(1)
            wdma(wk_st, wk_d)
            wdma(wv_st, wv_d)
            xdma(3)
            nc.gpsimd.dma_start(out=wo_st[:], in_=wo_d[:])
            xdma(5)
            xdma(7)
            nc.vector.tensor_copy(out=wq_sb[:], in_=wq_st[:])
            nc.vector.tensor_copy(out=wk_sb[:], in_=wk_st[:])
            nc.vector.tensor_copy(out=wv_sb[:], in_=wv_st[:])
            nc.vector.tensor_copy(out=wo_sb[:], in_=wo_st[:])

            def x_round(pool, tag, nks, warmup=False):
                L = {"A": 2048, "B": 1536, "V": 512}[tag]
                slab = pool.tile([P, L], FP, tag=tag)
                if warmup:
                    # ramp the PE p-state during the first DMA window so the
                    # transposes (and everything after) run at full clock
                    for _ in range(7):
                        nc.tensor.matmul(
                            slab[:, 0:P], lhsT=ident[:], rhs=ident[:],
                            start=True, stop=True, skip_group_check=True,
                        )
                for i, nk in enumerate(nks):
                    src_ = xt4s[nk // 4]
                    for cc in range(2):
                        nc.tensor.transpose(
                            slab[:, 256 * i + P * cc: 256 * i + P * (cc + 1)],
                            src_[:, 256 * (nk % 4) + P * cc: 256 * (nk % 4) + P * (cc + 1)],
                            ident[:],
                        )
                n = len(nks)
                sv = slab[:].rearrange("p (k c f) -> p k c f", c=2, f=P)
                h1 = n // 2
                for lo, hi in ((0, h1), (h1, n)):
                    nc.scalar.copy(
                        out=xT[0][:, P * (nks[0] + lo): P * (nks[0] + hi)],
                        in_=sv[:, lo:hi, 0, :],
                    )
                    nc.vector.tensor_copy(
                        out=xT[1][:, P * (nks[0] + lo): P * (nks[0] + hi)],
                        in_=sv[:, lo:hi, 1, :],
                    )

            def proj_slice(slab, w_sb, r, it):
                # slab[0:64, 512r:+512] = (x @ w)^T cols for i-tile `it`
                for cc in range(2):
                    nc.tensor.matmul(
                        slab[0:64, 512 * r: 512 * (r + 1)],
                        lhsT=w_sb[:, cc, :],
                        rhs=xT[cc][:, 512 * it: 512 * (it + 1)],
                        start=(cc == 0), stop=(cc == 1),
                    )

            def v_slices(slab, r0, k0, nk):
                # v chunks k0..k0+nk at slab cols 512*r0+
                for i in range(nk):
                    k = k0 + i
                    for cc in range(2):
                        nc.tensor.matmul(
                            slab[:, 512 * r0 + 64 * i: 512 * r0 + 64 * (i + 1)],
                            lhsT=xT[cc][:, P * k: P * (k + 1)],
                            rhs=wv_sb[:, cc, :],
                            start=(cc == 0), stop=(cc == 1),
                        )

            def v_copies(slab, r0, k0, nk):
                sv = slab[:, 512 * r0: 512 * r0 + 64 * nk].rearrange(
                    "p (k d) -> p k d", d=64
                )
                for h in range(2):
                    vv = vaug[h][:].rearrange("p (k e) -> p k e", e=33)
                    nc.vector.tensor_copy(
                        out=vv[:, k0:k0 + nk, 0:32],
                        in_=sv[:, 0:nk, 32 * h: 32 * (h + 1)],
                    )

            def kq_gen(pool, tag, slices, vpart=None, in_att=False):
                # slices: list of ("k"|"q", it). Pre-attention, karr copies
                # ride the idle Act engine; during attention Act is
                # exp-saturated so everything goes to DVE.
                L = 2048 if tag == "A" else 1536
                slab = pool.tile([P, L], FP, tag=tag)
                for r, (which, it) in enumerate(slices):
                    proj_slice(slab, wk_sb if which == "k" else wq_sb, r, it)
                if vpart is not None:
                    v_slices(slab, len(slices), vpart[0], vpart[1])
                for r, (which, it) in enumerate(slices):
                    dst = karr if which == "k" else qT
                    if which == "k" and not in_att and r % 2 == 0:
                        nc.scalar.copy(
                            out=dst[:, 512 * it: 512 * (it + 1)],
                            in_=slab[0:64, 512 * r: 512 * (r + 1)],
                        )
                    else:
                        nc.vector.tensor_copy(
                            out=dst[:, 512 * it: 512 * (it + 1)],
                            in_=slab[0:64, 512 * r: 512 * (r + 1)],
                        )
                if vpart is not None:
                    v_copies(slab, len(slices), vpart[0], vpart[1])

            def v_gen(pool, tag, k0, nk):
                L = 2048 if tag == "A" else 1536
                slab = pool.tile([P, L], FP, tag=tag)
                v_slices(slab, 0, k0, nk)
                v_copies(slab, 0, k0, nk)

            for h in range(2):
                vv = vaug[h][:].rearrange("p (k e) -> p k e", e=33)
                nc.vector.memset(vv[:, :, 32], 1.0)
            # pre-attention: x transposed (6 rounds over 3 psum rings to
            # break the same-ring relay), plus only what tile-0's early
            # groups need; the rest is built just-in-time inside
            # attention(0) via single-slice ring insertions that hide
            # under the opposite slab's exp.
            x_round(psA, "A", list(range(0, 8)), warmup=True)
            x_round(psB, "B", list(range(8, 14)))
            x_round(psV, "V", [14, 15])
            x_round(psA, "A", list(range(16, 24)))
            x_round(psB, "B", list(range(24, 30)))
            x_round(psV, "V", [30, 31])
            kq_gen(psA, "A", [("k", 0), ("q", 0), ("k", 1), ("k", 4)])
            kq_gen(psB, "B", [("k", 2), ("k", 3)], vpart=(0, 8))
            kq_gen(psB, "B", [("k", 5)], vpart=(8, 8))
            # ---- attention ----
            # per (h, it): sim slabs -> exp (fp16) -> attn@v accumulating
            # av[128i, 33]x4 blocks in the V bank (den in col 32); then
            # recip+scale (DVE), PE-transpose into outT[32h:+32, i-tile].
            # y(it-1) rides the B ring right after g7 (the B ring has two
            # A-exps of slack at each tile boundary, so this adds no Act
            # bubble); yo copies split DVE/Pool.
            def y_proj_half(it, pool, tag, m0, act_copy=False):
                # y blocks m0, m0+1 as a [P,512] gen on the given ring,
                # placed mid-tile where the chain (2 matmuls + copies)
                # hides under the other slab's exp.
                i0 = 512 * it
                yslab = pool.tile([P, 512], FP, tag=tag)
                for r in range(2):
                    m = m0 + r
                    nc.tensor.matmul(
                        yslab[:, 256 * r: 256 * (r + 1)],
                        lhsT=outT[0:64, i0 + P * m: i0 + P * (m + 1)],
                        rhs=wo_sb[:],
                        start=True, stop=True, skip_group_check=True,
                    )
                yo = ytmpp.tile([P, 512], FP, tag=f"yo{m0}")
                for r in range(2):
                    if act_copy and r == 0:
                        nc.scalar.copy(
                            out=yo[:, 256 * r: 256 * (r + 1)],
                            in_=yslab[:, 256 * r: 256 * (r + 1)],
                        )
                    else:
                        nc.vector.tensor_copy(
                            out=yo[:, 256 * r: 256 * (r + 1)],
                            in_=yslab[:, 256 * r: 256 * (r + 1)],
                        )
                    m = m0 + r
                    nc.sync.dma_start(
                        out=y_d[i0 + P * m: i0 + P * (m + 1), :],
                        in_=yo[:, 256 * r: 256 * (r + 1)],
                    )

            def attention(h, with_y):
                vv = vaug[h][:].rearrange("p (k e) -> p k e", e=33)
                tpos = None if h == 0 else (32, 0)
                for it in range(ITILES):
                    i0 = 512 * it
                    # V bank tile: cols 0-131 av (4 blocks x 33), 132-259
                    # transpose scratch; disjoint byte ranges within one gen
                    vt = psV.tile([P, 260], FP, tag="V")
                    avt = vt[:, 0:132]
                    av = avt.rearrange("p (m e) -> p m e", e=33)
                    cstart = 0
                    for gi, gsz in enumerate(GROUPS):
                        pool, tag = (psA, "A") if gsz != 3 else (psB, "B")
                        L = 512 * gsz
                        slab = pool.tile([P, L], FP, tag=tag)
                        for r in range(gsz):
                            c = cstart + r
                            nc.tensor.matmul(
                                slab[:, 512 * r: 512 * (r + 1)],
                                lhsT=karr[32 * h: 32 * (h + 1), P * c: P * (c + 1)],
                                rhs=qT[32 * h: 32 * (h + 1), i0: i0 + 512],
                                start=True, stop=True, tile_position=tpos,
                            )
                        eslab = expp.tile([P, L], F16, tag="E")
                        nc.scalar.activation(eslab[:], slab[:], AF.Exp)
                        for r in range(gsz):
                            c = cstart + r
                            for m in range(4):
                                nc.tensor.matmul(
                                    avt[:, 33 * m: 33 * (m + 1)],
                                    lhsT=eslab[:, 512 * r + P * m: 512 * r + P * (m + 1)],
                                    rhs=vv[:, c, :],
                                    start=(c == 0 and m == 0),
                                    stop=(c == NCH - 1 and m == 3),
                                    skip_group_check=True,
                                )
                        cstart += gsz
                        if h == 0 and it == 0:
                            if gi == 0:
                                v_gen(psA, "A", 16, 4)
                            elif gi == 1:
                                v_gen(psB, "B", 20, 4)
                            elif gi == 2:
                                kq_gen(psA, "A", [("k", 6)], in_att=True)
                            elif gi == 3:
                                v_gen(psB, "B", 24, 4)
                            elif gi == 4:
                                kq_gen(psA, "A", [("k", 7)], in_att=True)
                            elif gi == 5:
                                v_gen(psB, "B", 28, 4)
                            elif gi == 6:
                                kq_gen(psA, "A", [("q", 1)], in_att=True)
                        if gi == 8 and with_y and it > 0:
                            y_proj_half(it - 1, psV, "V", 0, act_copy=(it == 7))
                        if gi == 9 and with_y and it > 0:
                            y_proj_half(it - 1, psV, "V", 2, act_copy=(it == 7))
                        if gi == 9 and h == 0 and it < 6:
                            kq_gen(psB, "B", [("q", it + 2)], in_att=True)
                    # post: reciprocal of dens, normalize, transpose to
                    # outT. On the final tile the Act engine is already done
                    # with exps, so it takes half the copies.
                    last = h == 1 and it == ITILES - 1
                    rd = rden[:, 8 * it + 4 * h: 8 * it + 4 * h + 4]
                    nc.vector.reciprocal(out=rd, in_=av[:, :, 32])
                    stg = stgp.tile([P, P], FP, tag="s")
                    for m in range(4):
                        if last and m % 2 == 0:
                            nc.scalar.mul(
                                stg[:, 32 * m: 32 * (m + 1)], av[:, m, 0:32],
                                rd[:, m: m + 1],
                            )
                        else:
                            nc.vector.tensor_scalar_mul(
                                stg[:, 32 * m: 32 * (m + 1)], av[:, m, 0:32],
                                rd[:, m: m + 1],
                            )
                    nc.tensor.matmul(
                        vt[:, 132:260], lhsT=stg[:], rhs=ident[:],
                        is_transpose=True, start=True, stop=True,
                        skip_group_check=True,
                    )
                    for m in range(4):
                        if last and m % 2 == 0:
                            nc.scalar.copy(
                                out=outT[32 * h: 32 * h + 32, i0 + P * m: i0 + P * (m + 1)],
                                in_=vt[32 * m: 32 * (m + 1), 132:260],
                            )
                        else:
                            nc.vector.tensor_copy(
                                out=outT[32 * h: 32 * h + 32, i0 + P * m: i0 + P * (m + 1)],
                                in_=vt[32 * m: 32 * (m + 1), 132:260],
                            )

            attention(0, with_y=False)
            attention(1, with_y=True)
            y_proj_half(ITILES - 1, psV, "V", 0, act_copy=True)
            y_proj_half(ITILES - 1, psV, "V", 2, act_copy=True)

    _split_excess_waits(nc, mybir)
    return nc


def _split_excess_waits(nc, mybir, maxw=1, carrier_cap=1):
    """walrus codegen allows few semaphore waits per engine instruction.

    Tile's scheduler can emit 3-4 on one matmul. Hoist the excess onto
    InstEventSemaphore carriers inserted immediately before the instruction
    on the same engine queue (queue is FIFO, so waiting in the carrier is
    equivalent; no reordering so no deadlock risk).
    """
    skip = {
        "InstEventSemaphore", "InstCall",
        "InstUnconditionalBranch", "InstISA", "InstRegisterMove",
    }
    for f in nc.m.functions:
        for blk in f.blocks:
            idx = 0
            while idx < len(blk.instructions):
                ins = blk.instructions[idx]
                si = getattr(ins, "sync_info", None)
                if (
                    si is not None and si.on_wait and len(si.on_wait) > maxw
                    and type(ins).__name__ not in skip
                ):
                    waits = list(si.on_wait)
                    keep, excess = waits[:maxw], waits[maxw:]
                    n_ins = 0
                    for i in range(0, len(excess), carrier_cap):
                        ev = mybir.InstEventSemaphore(
                            name=nc.get_next_instruction_name(),
                            engine=ins.engine,
                            ins=[], outs=[],
                            sync_info=mybir.SyncInfo(
                                on_wait=excess[i:i + carrier_cap], on_update=[]
                            ),
                        )
                        nc.register_instruction(ev)
                        blk.instructions.insert(idx + n_ins, ev)
                        n_ins += 1
                    ins.sync_info = mybir.SyncInfo(
                        on_wait=keep, on_update=list(si.on_update or [])
                    )
                    idx += n_ins
                idx += 1
    return nc


def get_nc():
    if "nc" not in _CACHED:
        _CACHED["nc"] = _build_nc()
    return _CACHED["nc"]


def make_in_maps(x, w_qkv, w_out):
    """Host-side sharding: core c -> batch c//2, heads (c%2)*2, (c%2)*2+1."""
    B = x.shape[0]
    xf = np.ascontiguousarray(x.reshape(B, N, C))
    scale = DH ** -0.5
    in_maps = []
    for core in range(8):
        b, hp = core // 2, core % 2
        h0, h1 = 2 * hp, 2 * hp + 1
        wq = np.concatenate(
            [w_qkv[:, h * DH:(h + 1) * DH] * scale for h in (h0, h1)], axis=1
        )  # [256, 64]
        wk = np.concatenate(
            [w_qkv[:, 128 + h * DH: 128 + (h + 1) * DH] for h in (h0, h1)], axis=1
        )  # [256, 64]
        wv = np.concatenate(
            [w_qkv[:, 256 + h * DH: 256 + (h + 1) * DH] for h in (h0, h1)], axis=1
        )  # [256, 64]
        wo = np.concatenate(
            [w_out[h * DH:(h + 1) * DH, :] for h in (h0, h1)], axis=0
        )  # [64, 256]
        in_maps.append({
            "x": np.ascontiguousarray(xf[b]),
            "wq": np.ascontiguousarray(wq.astype(np.float32)),
            "wk": np.ascontiguousarray(wk.astype(np.float32)),
            "wv": np.ascontiguousarray(wv.astype(np.float32)),
            "wo": np.ascontiguousarray(wo.astype(np.float32)),
        })
    return in_maps


def kernel(x, w_qkv, w_out, b_out):
    from concourse.bass_utils import run_bass_kernel_spmd

    nc = get_nc()
    in_maps = make_in_maps(
        np.asarray(x, dtype=np.float32),
        np.asarray(w_qkv, dtype=np.float32),
        np.asarray(w_out, dtype=np.float32),
    )
    res = run_bass_kernel_spmd(nc, in_maps, list(range(8))).results
    B, H, W = 4, 64, 64
    y = np.empty((B, N, C), dtype=np.float32)
    for b in range(B):
        y[b] = res[2 * b]["y"] + res[2 * b + 1]["y"]
    y += np.asarray(b_out, dtype=np.float32)
    return y.reshape(B, H, W, C)


# revision 35
# speedup vs baseline: 1.0090x; 1.0090x over previous
"""Trainium2 Bass kernel for spatial self-attention (nn_Attention_90615220011343).

Module math (per batch b):
    qkv = x @ w_qkv            x:[N=4096, C=256], w_qkv:[256, 384]
    q,k,v -> heads (4 heads, dim 32)
    sim = (q*ds^-0.5) @ k^T    per head: [4096, 4096]
    attn = softmax(sim, -1)
    out = attn @ v             -> [N, 128]
    y = out @ w_out + b_out    -> [N, 256]

Sharding: 8 cores = 4 batches x 2 head-pairs. Core c -> batch c//2,
heads {2*(c%2), 2*(c%2)+1}. Each core computes a partial y (its two
heads' contribution); host sums the pair and adds b_out.

Per-core layout (all on-chip, no collectives). The Activation engine is
the roofline (33.5M softmax exps / 128 lanes); everything else is
organized to keep it saturated:
  - x^T [2x128, 4096] via PE transposes (contraction dim C on partitions).
  - q^T, k^T stored flat [32, 4096] fp32r (contract dim 32 on partitions
    0-31; sim matmuls stream 512-col i-tiles at 1 cycle/row).
  - sim^T computed in [j, i] psum slabs (A: 4 banks / B: 3 banks,
    ping-pong) so exp is one big Activation op per slab, no reductions.
  - exp output in fp16; attn@v flipped to out[i, d]: lhsT = exp-slab
    [j, 128-i-block], rhs = [v_h | 1] fp16 [j, 33] -> 33-row matmuls
    accumulating [128, 33] per i-block in psum, denominator rides in
    column 32.  4x fewer PE rows than the [d, i] orientation.
  - per i-tile: reciprocal + scale (DVE), PE-transpose of the normalized
    [128i, 4x32d] block into outT [64, 4096] fp16.
  - y = outT-block^T @ w_out contracts both heads at once (fp16), riding
    the A-slab psum ring lagged one i-tile behind attention.
"""

import numpy as np

HEADS = 4
DH = 32
N = 4096
C = 256
P = 128
NCH = 32  # number of 128-token j-chunks
ITILES = 8  # i tiles of 512
# j-chunks per sim/exp group. The 1-chunk A group before the final B group
# keeps every same-slab pair separated by the other slab's exp, so sim
# refills always hide under an exp (no tile-boundary fill gap).
GROUPS = [4, 3, 4, 3, 4, 3, 4, 3, 1, 3]

_CACHED = {}


def _build_nc():
    import concourse.bass as bass
    import concourse.mybir as mybir
    from concourse.tile import TileContext
    from concourse.masks import make_identity

    FP = mybir.dt.float32
    FR = mybir.dt.float32r
    F16 = mybir.dt.float16
    AF = mybir.ActivationFunctionType

    nc = bass.Bass(target_bir_lowering=False)
    x_d = nc.declare_dram_parameter("x", [N, C], FP, isOutput=False)
    wq_d = nc.declare_dram_parameter("wq", [C, 64], FP, isOutput=False)
    wk_d = nc.declare_dram_parameter("wk", [C, 64], FP, isOutput=False)
    wv_d = nc.declare_dram_parameter("wv", [C, 64], FP, isOutput=False)
    wo_d = nc.declare_dram_parameter("wo", [64, C], FP, isOutput=False)
    y_d = nc.declare_dram_parameter("y", [N, C], FP, isOutput=True)

    with TileContext(nc) as tc:
        with (
            tc.tile_pool(name="const", bufs=1) as constp,
            tc.tile_pool(name="xin", bufs=1) as xinp,
            tc.tile_pool(name="big", bufs=1) as bigp,
            tc.tile_pool(name="exp", bufs=3) as expp,
            tc.tile_pool(name="stg", bufs=2) as stgp,
            tc.tile_pool(name="ytmp", bufs=2) as ytmpp,
            tc.tile_pool(name="psA", bufs=1, space="PSUM") as psA,
            tc.tile_pool(name="psB", bufs=1, space="PSUM") as psB,
            tc.tile_pool(name="psV", bufs=1, space="PSUM") as psV,
        ):
            ident = constp.tile([P, P], FP, tag="ident")
            make_identity(nc, ident[:])
            # preload the activation table so the first real Act op (x copy
            # or exp) doesn't pay the 1.3us table load mid-chain
            warm = constp.tile([P, 1], FP, tag="warm")
            nc.vector.memset(warm[:], 0.0)
            nc.scalar.activation(warm[:], warm[:], AF.Exp)

            # ---- persistent SBUF tensors ----
            xT = [bigp.tile([P, N], FR, tag=f"xT{cc}", name=f"xT{cc}") for cc in range(2)]
            qT = bigp.tile([64, N], FR, tag="qT")
            karr = bigp.tile([64, N], FR, tag="karr")
            vaug = [bigp.tile([P, 33 * NCH], F16, tag=f"vaug{h}", name=f"vaug{h}") for h in range(2)]
            outT = bigp.tile([64, N], F16, tag="outT")
            rden = bigp.tile([P, 8 * ITILES], FP, tag="rden")
            wq_sb = bigp.tile([P, 2, 64], FR, tag="wq")
            wk_sb = bigp.tile([P, 2, 64], FR, tag="wk")
            wv_sb = bigp.tile([P, 2, 64], FR, tag="wv")
            wo_sb = bigp.tile([64, C], F16, tag="wo")

            # ---- weight staging tiles (DMAs interleaved with x below) ----
            wq_st = bigp.tile([P, 2, 64], FP, tag="wq_st")
            wk_st = bigp.tile([P, 2, 64], FP, tag="wk_st")
            wv_st = bigp.tile([P, 2, 64], FP, tag="wv_st")
            wo_st = bigp.tile([64, C], FP, tag="wo_st")

            def wdma(st, d):
                nc.gpsimd.dma_start(
                    out=st[:], in_=d[:, :].rearrange("(c p) f -> p c f", p=P)
                )

            # ---- x load + transpose to xT; qkv builds interleaved ----
            # 8 batched DMAs (4 chunks each) into persistent staging tiles;
            # build generations are interleaved into the A/B psum rings so
            # each gen's ring predecessor matches its data dependencies.
            # karr copies ride the idle Act engine, qT/v copies ride DVE.
            xt4s = [
                xinp.tile([P, 4 * C], FP, tag=f"xt{b}", name=f"xt4_{b}")
                for b in range(8)
            ]

            def xdma(b):
                dmae = nc.sync if b % 2 == 0 else nc.gpsimd
                dmae.dma_start(
                    out=xt4s[b][:].rearrange("p (k c) -> p k c", c=C),
                    in_=x_d[512 * b: 512 * (b + 1), :].rearrange(
                        "(k p) c -> p k c", p=P
                    ),
                )

            for b in (0, 2, 4, 6):
                xdma(b)
            wdma(wq_st, wq_d)
            xdma(1)
            wdma(wk_st, wk_d)
            wdma(wv_st, wv_d)
            xdma(3)
            nc.gpsimd.dma_start(out=wo_st[:], in_=wo_d[:])
            xdma(5)
            xdma(7)
            nc.vector.tensor_copy(out=wq_sb[:], in_=wq_st[:])
            nc.vector.tensor_copy(out=wk_sb[:], in_=wk_st[:])
            nc.vector.tensor_copy(out=wv_sb[:], in_=wv_st[:])
            nc.vector.tensor_copy(out=wo_sb[:], in_=wo_st[:])

            def x_round(pool, tag, nks, warmup=False):
                L = {"A": 2048, "B": 1536, "V": 512}[tag]
                slab = pool.tile([P, L], FP, tag=tag)
                if warmup:
                    # ramp the PE p-state during the first DMA window so the
                    # transposes (and everything after) run at full clock
                    for _ in range(7):
                        nc.tensor.matmul(
                            slab[:, 0:P], lhsT=ident[:], rhs=ident[:],
                            start=True, stop=True, skip_group_check=True,
                        )
                for i, nk in enumerate(nks):
                    src_ = xt4s[nk // 4]
                    for cc in range(2):
                        nc.tensor.transpose(
                            slab[:, 256 * i + P * cc: 256 * i + P * (cc + 1)],
                            src_[:, 256 * (nk % 4) + P * cc: 256 * (nk % 4) + P * (cc + 1)],
                            ident[:],
                        )
                n = len(nks)
                sv = slab[:].rearrange("p (k c f) -> p k c f", c=2, f=P)
                h1 = n // 2
                for lo, hi in ((0, h1), (h1, n)):
                    nc.scalar.copy(
                        out=xT[0][:, P * (nks[0] + lo): P * (nks[0] + hi)],
                        in_=sv[:, lo:hi, 0, :],
                    )
                    nc.vector.tensor_copy(
                        out=xT[1][:, P * (nks[0] + lo): P * (nks[0] + hi)],
                        in_=sv[:, lo:hi, 1, :],
                    )

            def proj_slice(slab, w_sb, r, it):
                # slab[0:64, 512r:+512] = (x @ w)^T cols for i-tile `it`
                for cc in range(2):
                    nc.tensor.matmul(
                        slab[0:64, 512 * r: 512 * (r + 1)],
                        lhsT=w_sb[:, cc, :],
                        rhs=xT[cc][:, 512 * it: 512 * (it + 1)],
                        start=(cc == 0), stop=(cc == 1),
                    )

            def v_slices(slab, r0, k0, nk):
                # v chunks k0..k0+nk at slab cols 512*r0+
                for i in range(nk):
                    k = k0 + i
                    for cc in range(2):
                        nc.tensor.matmul(
                            slab[:, 512 * r0 + 64 * i: 512 * r0 + 64 * (i + 1)],
                            lhsT=xT[cc][:, P * k: P * (k + 1)],
                            rhs=wv_sb[:, cc, :],
                            start=(cc == 0), stop=(cc == 1),
                        )

            def v_copies(slab, r0, k0, nk):
                sv = slab[:, 512 * r0: 512 * r0 + 64 * nk].rearrange(
                    "p (k d) -> p k d", d=64
                )
                for h in range(2):
                    vv = vaug[h][:].rearrange("p (k e) -> p k e", e=33)
                    nc.vector.tensor_copy(
                        out=vv[:, k0:k0 + nk, 0:32],
                        in_=sv[:, 0:nk, 32 * h: 32 * (h + 1)],
                    )

            def kq_gen(pool, tag, slices, vpart=None, in_att=False):
                # slices: list of ("k"|"q", it). Pre-attention, karr copies
                # ride the idle Act engine; during attention Act is
                # exp-saturated so everything goes to DVE.
                L = 2048 if tag == "A" else 1536
                slab = pool.tile([P, L], FP, tag=tag)
                for r, (which, it) in enumerate(slices):
                    proj_slice(slab, wk_sb if which == "k" else wq_sb, r, it)
                if vpart is not None:
                    v_slices(slab, len(slices), vpart[0], vpart[1])
                for r, (which, it) in enumerate(slices):
                    dst = karr if which == "k" else qT
                    if which == "k" and not in_att and r % 2 == 0:
                        nc.scalar.copy(
                            out=dst[:, 512 * it: 512 * (it + 1)],
                            in_=slab[0:64, 512 * r: 512 * (r + 1)],
                        )
                    else:
                        nc.vector.tensor_copy(
                            out=dst[:, 512 * it: 512 * (it + 1)],
                            in_=slab[0:64, 512 * r: 512 * (r + 1)],
                        )
                if vpart is not None:
                    v_copies(slab, len(slices), vpart[0], vpart[1])

            def v_gen(pool, tag, k0, nk):
                L = 2048 if tag == "A" else 1536
                slab = pool.tile([P, L], FP, tag=tag)
                v_slices(slab, 0, k0, nk)
                v_copies(slab, 0, k0, nk)

            for h in range(2):
                vv = vaug[h][:].rearrange("p (k e) -> p k e", e=33)
                nc.vector.memset(vv[:, :, 32], 1.0)
            # pre-attention: x transposed (6 rounds over 3 psum rings to
            # break the same-ring relay), plus only what tile-0's early
            # groups need; the rest is built just-in-time inside
            # attention(0) via single-slice ring insertions that hide
            # under the opposite slab's exp.
            x_round(psA, "A", list(range(0, 8)), warmup=True)
            x_round(psB, "B", list(range(8, 14)))
            x_round(psV, "V", [14, 15])
            x_round(psA, "A", list(range(16, 24)))
            x_round(psB, "B", list(range(24, 30)))
            x_round(psV, "V", [30, 31])
            kq_gen(psA, "A", [("k", 0), ("q", 0), ("k", 1), ("k", 4)])
            kq_gen(psB, "B", [("k", 2), ("k", 3)], vpart=(0, 8))
            kq_gen(psB, "B", [("k", 5)], vpart=(8, 8))
            # ---- attention ----
            # per (h, it): sim slabs -> exp (fp16) -> attn@v accumulating
            # av[128i, 33]x4 blocks in the V bank (den in col 32); then
            # recip+scale (DVE), PE-transpose into outT[32h:+32, i-tile].
            # y(it-1) rides the B ring right after g7 (the B ring has two
            # A-exps of slack at each tile boundary, so this adds no Act
            # bubble); yo copies split DVE/Pool.
            def y_proj_half(it, pool, tag, m0, act_copy=False):
                # y blocks m0, m0+1 as a [P,512] gen on the given ring,
                # placed mid-tile where the chain (2 matmuls + copies)
                # hides under the other slab's exp.
                i0 = 512 * it
                yslab = pool.tile([P, 512], FP, tag=tag)
                for r in range(2):
                    m = m0 + r
                    nc.tensor.matmul(
                        yslab[:, 256 * r: 256 * (r + 1)],
                        lhsT=outT[0:64, i0 + P * m: i0 + P * (m + 1)],
                        rhs=wo_sb[:],
                        start=True, stop=True, skip_group_check=True,
                    )
                yo = ytmpp.tile([P, 512], FP, tag=f"yo{m0}")
                for r in range(2):
                    if act_copy and r == 0:
                        nc.scalar.copy(
                            out=yo[:, 256 * r: 256 * (r + 1)],
                            in_=yslab[:, 256 * r: 256 * (r + 1)],
                        )
                    else:
                        nc.vector.tensor_copy(
                            out=yo[:, 256 * r: 256 * (r + 1)],
                            in_=yslab[:, 256 * r: 256 * (r + 1)],
                        )
                    m = m0 + r
                    nc.sync.dma_start(
                        out=y_d[i0 + P * m: i0 + P * (m + 1), :],
                        in_=yo[:, 256 * r: 256 * (r + 1)],
                    )

            def attention(h, with_y):
                vv = vaug[h][:].rearrange("p (k e) -> p k e", e=33)
                tpos = None if h == 0 else (32, 0)
                for it in range(ITILES):
                    i0 = 512 * it
                    # V bank tile: cols 0-131 av (4 blocks x 33), 132-259
                    # transpose scratch; disjoint byte ranges within one gen
                    vt = psV.tile([P, 260], FP, tag="V")
                    avt = vt[:, 0:132]
                    av = avt.rearrange("p (m e) -> p m e", e=33)
                    cstart = 0
                    for gi, gsz in enumerate(GROUPS):
                        pool, tag = (psA, "A") if gsz != 3 else (psB, "B")
                        L = 512 * gsz
                        slab = pool.tile([P, L], FP, tag=tag)
                        for r in range(gsz):
                            c = cstart + r
                            nc.tensor.matmul(
                                slab[:, 512 * r: 512 * (r + 1)],
                                lhsT=karr[32 * h: 32 * (h + 1), P * c: P * (c + 1)],
                                rhs=qT[32 * h: 32 * (h + 1), i0: i0 + 512],
                                start=True, stop=True, tile_position=tpos,
                            )
                        eslab = expp.tile([P, L], F16, tag="E")
                        nc.scalar.activation(eslab[:], slab[:], AF.Exp)
                        for r in range(gsz):
                            c = cstart + r
                            for m in range(4):
                                nc.tensor.matmul(
                                    avt[:, 33 * m: 33 * (m + 1)],
                                    lhsT=eslab[:, 512 * r + P * m: 512 * r + P * (m + 1)],
                                    rhs=vv[:, c, :],
                                    start=(c == 0 and m == 0),
                                    stop=(c == NCH - 1 and m == 3),
                                    skip_group_check=True,
                                )
                        cstart += gsz
                        if h == 0 and it == 0:
                            if gi == 0:
                                v_gen(psA, "A", 16, 4)
                            elif gi == 1:
                                v_gen(psB, "B", 20, 4)
                            elif gi == 2:
                                kq_gen(psA, "A", [("k", 6)], in_att=True)
                            elif gi == 3:
                                v_gen(psB, "B", 24, 4)
                            elif gi == 4:
                                kq_gen(psA, "A", [("k", 7)], in_att=True)
                            elif gi == 5:
                                v_gen(psB, "B", 28, 4)
                            elif gi == 6:
                                kq_gen(psA, "A", [("q", 1)], in_att=True)
                        if gi == 8 and with_y and it > 0:
                            y_proj_half(it - 1, psA, "A", 0, act_copy=(it == 7))
                        if gi == 9 and with_y and it > 0:
                            y_proj_half(it - 1, psB, "B", 2, act_copy=(it == 7))
                        if gi == 9 and h == 0 and it < 6:
                            kq_gen(psB, "B", [("q", it + 2)], in_att=True)
                    # post: reciprocal of dens, normalize, transpose to
                    # outT. On the final tile the Act engine is already done
                    # with exps, so it takes half the copies.
                    last = h == 1 and it == ITILES - 1
                    rd = rden[:, 8 * it + 4 * h: 8 * it + 4 * h + 4]
                    nc.vector.reciprocal(out=rd, in_=av[:, :, 32])
                    stg = stgp.tile([P, P], FP, tag="s")
                    for m in range(4):
                        if last and m % 2 == 0:
                            nc.scalar.mul(
                                stg[:, 32 * m: 32 * (m + 1)], av[:, m, 0:32],
                                rd[:, m: m + 1],
                            )
                        else:
                            nc.vector.tensor_scalar_mul(
                                stg[:, 32 * m: 32 * (m + 1)], av[:, m, 0:32],
                                rd[:, m: m + 1],
                            )
                    nc.tensor.matmul(
                        vt[:, 132:260], lhsT=stg[:], rhs=ident[:],
                        is_transpose=True, start=True, stop=True,
                        skip_group_check=True,
                    )
                    for m in range(4):
                        if last and m % 2 == 0:
                            nc.scalar.copy(
                                out=outT[32 * h: 32 * h + 32, i0 + P * m: i0 + P * (m + 1)],
                                in_=vt[32 * m: 32 * (m + 1), 132:260],
                            )
                        else:
                            nc.vector.tensor_copy(
                                out=outT[32 * h: 32 * h + 32, i0 + P * m: i0 + P * (m + 1)],
                                in_=vt[32 * m: 32 * (m + 1), 132:260],
                            )

            attention(0, with_y=False)
            attention(1, with_y=True)
            y_proj_half(ITILES - 1, psA, "A", 0, act_copy=True)
            y_proj_half(ITILES - 1, psB, "B", 2, act_copy=True)

    _split_excess_waits(nc, mybir)
    return nc


def _split_excess_waits(nc, mybir, maxw=1, carrier_cap=1):
    """walrus codegen allows few semaphore waits per engine instruction.

    Tile's scheduler can emit 3-4 on one matmul. Hoist the excess onto
    InstEventSemaphore carriers inserted immediately before the instruction
    on the same engine queue (queue is FIFO, so waiting in the carrier is
    equivalent; no reordering so no deadlock risk).
    """
    skip = {
        "InstEventSemaphore", "InstCall",
        "InstUnconditionalBranch", "InstISA", "InstRegisterMove",
    }
    for f in nc.m.functions:
        for blk in f.blocks:
            idx = 0
            while idx < len(blk.instructions):
                ins = blk.instructions[idx]
                si = getattr(ins, "sync_info", None)
                if (
                    si is not None and si.on_wait and len(si.on_wait) > maxw
                    and type(ins).__name__ not in skip
                ):
                    waits = list(si.on_wait)
                    keep, excess = waits[:maxw], waits[maxw:]
                    n_ins = 0
                    for i in range(0, len(excess), carrier_cap):
                        ev = mybir.InstEventSemaphore(
                            name=nc.get_next_instruction_name(),
                            engine=ins.engine,
                            ins=[], outs=[],
                            sync_info=mybir.SyncInfo(
                                on_wait=excess[i:i + carrier_cap], on_update=[]
                            ),
                        )
                        nc.register_instruction(ev)
                        blk.instructions.insert(idx + n_ins, ev)
                        n_ins += 1
                    ins.sync_info = mybir.SyncInfo(
                        on_wait=keep, on_update=list(si.on_update or [])
                    )
                    idx += n_ins
                idx += 1
    return nc


def get_nc():
    if "nc" not in _CACHED:
        _CACHED["nc"] = _build_nc()
    return _CACHED["nc"]


def make_in_maps(x, w_qkv, w_out):
    """Host-side sharding: core c -> batch c//2, heads (c%2)*2, (c%2)*2+1."""
    B = x.shape[0]
    xf = np.ascontiguousarray(x.reshape(B, N, C))
    scale = DH ** -0.5
    in_maps = []
    for core in range(8):
        b, hp = core // 2, core % 2
        h0, h1 = 2 * hp, 2 * hp + 1
        wq = np.concatenate(
            [w_qkv[:, h * DH:(h + 1) * DH] * scale for h in (h0, h1)], axis=1
        )  # [256, 64]
        wk = np.concatenate(
            [w_qkv[:, 128 + h * DH: 128 + (h + 1) * DH] for h in (h0, h1)], axis=1
        )  # [256, 64]
        wv = np.concatenate(
            [w_qkv[:, 256 + h * DH: 256 + (h + 1) * DH] for h in (h0, h1)], axis=1
        )  # [256, 64]
        wo = np.concatenate(
            [w_out[h * DH:(h + 1) * DH, :] for h in (h0, h1)], axis=0
        )  # [64, 256]
        in_maps.append({
            "x": np.ascontiguousarray(xf[b]),
            "wq": np.ascontiguousarray(wq.astype(np.float32)),
            "wk": np.ascontiguousarray(wk.astype(np.float32)),
            "wv": np.ascontiguousarray(wv.astype(np.float32)),
            "wo": np.ascontiguousarray(wo.astype(np.float32)),
        })
    return in_maps


def kernel(x, w_qkv, w_out, b_out):
    from concourse.bass_utils import run_bass_kernel_spmd

    nc = get_nc()
    in_maps = make_in_maps(
        np.asarray(x, dtype=np.float32),
        np.asarray(w_qkv, dtype=np.float32),
        np.asarray(w_out, dtype=np.float32),
    )
    res = run_bass_kernel_spmd(nc, in_maps, list(range(8))).results
    B, H, W = 4, 64, 64
    y = np.empty((B, N, C), dtype=np.float32)
    for b in range(B):
        y[b] = res[2 * b]["y"] + res[2 * b + 1]["y"]
    y += np.asarray(b_out, dtype=np.float32)
    return y.reshape(B, H, W, C)
